# revision 2
# baseline (speedup 1.0000x reference)
"""Multi-head attention forward on 8 Trainium2 NeuronCores (Bass/Tile), v2.

Problem: nn_MultiHeadAttention — B=8, T=1024, C=768, H=12, D=64, fp32 in/out.

Sharding: data-parallel over batch — one batch element per core; weights
broadcast (each core loads its own copy). No collectives. Host pre-transposes
x[b] to x^T [C, T] and converts x/W to bf16; fp32 PSUM accumulation keeps the
final rel err ~7e-3 (tolerance 2e-2).

Per-core kernel, all matmul operands bf16 (PE streams 1 col/cycle, same as
f32r, but: FWL weight loads, 2x ScalarE exp via bf16 output, half the DMA/SBUF
footprint):
  1. V = x @ Wv -> V_aug [128, T/128, H, 65] bf16 with a ones column per head
     (the ones row of att@V yields the softmax denominator for free).
  2. Per head pair p: Q^T/K^T chunks [128, T] (pair-major: head A on
     partitions 0-63, head B on 64-127) via matmul(lhsT=W[:,co], rhs=xT).
  3. S^T per pair via ROW-TILED matmuls: tile (0,0) contracts partitions 0-63
     (head A, K=64), tile (64,0) contracts 64-127 (head B) — both run
     CONCURRENTLY on the PE (measured 246ns per 512-col pair vs 430ns for the
     zero-padded K=128 scheme). pt = exp(S^T/8) on ScalarE with bf16 output
     (measured 560ns vs 1129ns for fp32 output on [128,1024]).
  4. att@V: Ytil[65, i] += V_aug[:, j, h, :]^T @ pt chunks. Head A consumes
     pt_A immediately; head B's pt is kept in SBUF and its att@V runs during
     the NEXT pair's (ScalarE-bound) S/exp phase, time-sharing its PSUM slot
     with the next pair's Q/K projection (PSUM: 4 banks S^T + 2 psyA + 2
     shared = exactly 8).
  5. Normalization at Ytil evacuation: denominator row -> ScalarE copy to a
     partition-0 staging row, DVE reciprocal_approx_fast, GpSimd
     partition_broadcast across the 64 d-rows, one DVE multiply into YT
     (bf16). The output projection then only needs a plain bias add.
"""
import numpy as np

B, T, C = 8, 1024, 768
H, D = 12, 64
P = 128
KS = C // P          # 6 contraction subtiles
TS = T // P          # 8 t subtiles
NP = H // 2          # 6 head pairs
N_CORES = 8

_RUNNER_CACHE = {}


def build_nc(reps: int = 1, phases: int = 4, npairs: int = NP):
    import concourse.bacc as bacc
    import concourse.mybir as mybir
    import concourse.tile as tile
    from contextlib import ExitStack

    f32 = mybir.dt.float32
    bf16 = mybir.dt.bfloat16
    AF = mybir.ActivationFunctionType
    ALU = mybir.AluOpType

    nc = bacc.Bacc(num_devices=N_CORES)

    xT_d = nc.dram_tensor("xT", [C, T], bf16, kind="ExternalInput")
    W_d = {w: nc.dram_tensor(f"W{w}", [C, C], bf16, kind="ExternalInput")
           for w in ("q", "k", "v", "p")}
    bqT_d = nc.dram_tensor("bqT", [P, KS], f32, kind="ExternalInput")
    bkT_d = nc.dram_tensor("bkT", [P, KS], f32, kind="ExternalInput")
    bvB_d = nc.dram_tensor("bvB", [P, C], f32, kind="ExternalInput")
    bpB_d = nc.dram_tensor("bpB", [P, C], f32, kind="ExternalInput")
    y_d = nc.dram_tensor("y", [T, C], f32, kind="ExternalOutput")

    with tile.TileContext(nc) as tc, ExitStack() as ctx:
        const = ctx.enter_context(tc.tile_pool(name="const", bufs=1))
        qkp = ctx.enter_context(tc.tile_pool(name="qk", bufs=1))
        ptAp = ctx.enter_context(tc.tile_pool(name="ptA", bufs=1))
        ptBp = ctx.enter_context(tc.tile_pool(name="ptB", bufs=1))
        opool = ctx.enter_context(tc.tile_pool(name="out", bufs=1))
        psS = ctx.enter_context(tc.tile_pool(name="psS", bufs=1, space="PSUM"))
        psY = ctx.enter_context(tc.tile_pool(name="psY", bufs=1, space="PSUM"))
        psX = ctx.enter_context(tc.tile_pool(name="psX", bufs=1, space="PSUM"))

        def body(_iv=None):
            # ---- loads ----
            xTr = const.tile([P, KS, T], bf16, tag="xT", name="xTr")
            Wr = {}
            for w in ("q", "k", "v"):
                Wr[w] = const.tile([P, KS, C], bf16, tag=f"W{w}", name=f"W{w}r")
            xT_r = xT_d.rearrange("(ks p) t -> p ks t", p=P)
            W_r = {w: W_d[w].rearrange("(ks p) c -> p ks c", p=P)
                   for w in ("q", "k", "v", "p")}
            # one DMA per tensor: each dma_start has ~1-2us fixed completion
            # latency on the HWDGE ring, so few big transfers beat many slices.
            # Big tensors first (they gate V-proj/QK-proj), biases after.
            nc.sync.dma_start(Wr["v"][:], W_r["v"])
            nc.sync.dma_start(xTr[:], xT_r)
            bvB = const.tile([P, C], f32, tag="bvB", name="bvB")
            nc.sync.dma_start(bvB[:], bvB_d[:, :])
            nc.sync.dma_start(Wr["q"][:], W_r["q"])
            nc.sync.dma_start(Wr["k"][:], W_r["k"])
            bqT = const.tile([P, KS], f32, tag="bqT", name="bqT")
            nc.sync.dma_start(bqT[:], bqT_d[:, :])
            bkT = const.tile([P, KS], f32, tag="bkT", name="bkT")
            nc.sync.dma_start(bkT[:], bkT_d[:, :])
            bpB = const.tile([P, C], f32, tag="bpB", name="bpB")
            nc.sync.dma_start(bpB[:], bpB_d[:, :])

            ones1 = const.tile([P, 1], f32, tag="ones", name="ones1")
            nc.vector.memset(ones1[:], 1.0)

            # ---- V projection into V_aug (bf16) with ones column ----
            V_aug = const.tile([P, TS, H, D + 1], bf16, tag="Vaug", name="Vaug")
            nc.vector.tensor_copy(V_aug[:, :, :, D:D + 1],
                                  ones1[:].to_broadcast([P, TS, H, 1]))
            for ts_ in range(TS):
                psv = psS.tile([P, 1024], f32, tag=f"S{ts_ % 2}", name="psv")
                for k in range(KS):
                    lhsT = xTr[:, k, ts_ * P:(ts_ + 1) * P]
                    nc.tensor.matmul(psv[:, 0:512], lhsT, Wr["v"][:, k, 0:512],
                                     start=(k == 0), stop=(k == KS - 1))
                    nc.tensor.matmul(psv[:, 512:768], lhsT, Wr["v"][:, k, 512:768],
                                     start=(k == 0), stop=(k == KS - 1))
                nc.vector.tensor_tensor(
                    V_aug[:, ts_, :, 0:D],
                    psv[:, 0:768].rearrange("p (h d) -> p h d", h=H),
                    bvB[:].rearrange("p (h d) -> p h d", h=H), op=ALU.add)

            if phases < 3:
                YTdummy = opool.tile([P, C], f32, tag="ot", name="ytd")
                nc.vector.memset(YTdummy[:], 0.0)
                nc.sync.dma_start(y_d[0:P, :], YTdummy[:])
                return

            YT = const.tile([P, KS, T], bf16, tag="YTs", name="YT")

            def emit_qk_proj(p, which):
                """Q or K projection for pair p into qkp tile (bf16 [128,T])."""
                w = "q" if which == "Q" else "k"
                bias = bqT if which == "Q" else bkT
                ps = psX.tile([P, 1024], f32, tag="aux", name=f"ps{which}")
                for ih in range(2):
                    for k in range(KS):
                        nc.tensor.matmul(
                            ps[:, ih * 512:(ih + 1) * 512],
                            Wr[w][:, k, p * P:(p + 1) * P],
                            xTr[:, k, ih * 512:(ih + 1) * 512],
                            start=(k == 0), stop=(k == KS - 1))
                out = qkp.tile([P, T], bf16, tag=f"{which}{p % 2}", name=f"{which}T2")
                nc.vector.tensor_tensor(
                    out[:], ps[:], bias[:, p:p + 1].to_broadcast([P, T]),
                    op=ALU.add)
                return out

            def evac_copy(p, hh, psy):
                """Fast PSUM->SBUF evacuation: Ytil rows via DVE, denominator
                row via ScalarE to a partition-0 staging row (custom-DVE recip
                needs base partition 0). These are the only psy readers, so the
                accumulator banks free after ~1.2us instead of the whole
                normalize chain."""
                h = 2 * p + hh
                tmp = qkp.tile([P, T], f32, tag=f"tmp{hh}{p % 2}", name="tmp")
                nc.vector.tensor_copy(tmp[0:D, :], psy[0:D, :])
                dst = qkp.tile([1, T], f32, tag=f"dst{h % 2}", name="dstage")
                nc.scalar.copy(dst[:], psy[D:D + 1, :])
                return tmp, dst

            def evac_recip(p, hh, dst):
                """Reciprocal + partition-broadcast of the denominator row; the
                GpSimd broadcast (~1.5us) runs while the PE continues, and the
                dependent multiply is emitted much later so the DVE FIFO never
                blocks on it."""
                h = 2 * p + hh
                rcp = qkp.tile([1, T], f32, tag=f"rcp{h % 2}", name="rcp")
                nc.vector.reciprocal_approx_fast(rcp[:], dst[:])
                rb = qkp.tile([D, T], f32, tag=f"rb{h % 2}", name="rb")
                nc.gpsimd.partition_broadcast(rb[:], rcp[:])
                return rb

            def evac_mult(p, hh, tmp, rb):
                b0 = 64 * hh
                nc.vector.tensor_tensor(YT[b0:b0 + 64, p, :], tmp[0:D, :], rb[:],
                                        op=ALU.mult)

            # ---- attention pair loop (software-pipelined) ----
            # Per j step the PE emits: S^T(j) [4 row-tiled MMs], attV_A(j-1)
            # [2 MMs, one j behind so exp(j-1) is long done], 2 chunks of the
            # PREVIOUS pair's head-B att@V on js 0-3 (its PSUM slot then hands
            # over to the next pair's Q/K projection MMs on js 4-7). The PE
            # never FIFO-blocks on a fresh exp result.
            def emit_qk_mms(p, which, ps, kslice):
                w = "q" if which == "Q" else "k"
                for ih in range(2):
                    for k in kslice:
                        nc.tensor.matmul(
                            ps[:, ih * 512:(ih + 1) * 512],
                            Wr[w][:, k, p * P:(p + 1) * P],
                            xTr[:, k, ih * 512:(ih + 1) * 512],
                            start=(k == 0), stop=(k == KS - 1))

            def emit_qk_evac(p, which, ps):
                bias = bqT if which == "Q" else bkT
                out = qkp.tile([P, T], bf16, tag=f"{which}{p % 2}", name=f"{which}T2")
                nc.vector.tensor_tensor(
                    out[:], ps[:], bias[:, p:p + 1].to_broadcast([P, T]),
                    op=ALU.add)
                return out

            def emit_qk_proj(p, which):
                ps = psX.tile([P, 1024], f32, tag="aux", name=f"ps{which}")
                emit_qk_mms(p, which, ps, range(KS))
                return emit_qk_evac(p, which, ps)

            def attv(psy, v_slice, pt, j):
                for ih in range(2):
                    sl = slice(ih * 512, (ih + 1) * 512)
                    nc.tensor.matmul(psy[0:D + 1, sl], v_slice, pt[:, sl],
                                     start=(j == 0), stop=(j == TS - 1))

            qt_cur = emit_qk_proj(0, "Q")
            kt_cur = emit_qk_proj(0, "K")
            tmpA_prev = tmpB_prev = None
            psyA_prev = None
            qt_nxt = kt_nxt = None
            ptB_tiles = [None] * TS
            ptB_prev = [None] * TS

            for p in range(npairs):
                last = (p == NP - 1)
                # free the previous pair's accumulator with one fast copy,
                # then normalize off the critical path
                if psyA_prev is not None:
                    tmpA_prev = evac_copy(p - 1, 0, psyA_prev)
                psyA = psY.tile([P, 1024], f32, tag="psyA", name="psyA")
                rbA = None
                if tmpA_prev is not None:
                    rbA = evac_recip(p - 1, 0, tmpA_prev[1])

                prevB = None
                if p > 0:
                    prevB = psX.tile([P, 1024], f32, tag="aux", name="psyBprev")
                psyB = None
                ptA_tiles = [None] * TS

                qps = kps = None
                for j in range(TS):
                    psA = psS.tile([P, 1024], f32, tag="S0", name="psA")
                    psB = psS.tile([P, 1024], f32, tag="S1", name="psB")
                    for ih in range(2):
                        sl = slice(ih * 512, (ih + 1) * 512)
                        nc.tensor.matmul(psA[:, sl],
                                         kt_cur[0:64, j * P:(j + 1) * P],
                                         qt_cur[0:64, sl],
                                         start=True, stop=True,
                                         tile_position=(0, 0))
                        nc.tensor.matmul(psB[:, sl],
                                         kt_cur[64:128, j * P:(j + 1) * P],
                                         qt_cur[64:128, sl],
                                         start=True, stop=True,
                                         tile_position=(64, 0))
                    ptA = ptAp.tile([P, 1024], bf16, tag=f"ptA{j % 4}", name="ptA")
                    nc.scalar.activation(ptA[:], psA[:], AF.Exp, scale=0.125)
                    ptA_tiles[j] = ptA
                    ptB = ptBp.tile([P, 1024], bf16, tag=f"ptB{j}", name="ptB")
                    nc.scalar.activation(ptB[:], psB[:], AF.Exp, scale=0.125)
                    ptB_tiles[j] = ptB

                    # attV_A one j behind: exp(j-1) finished during S(j)
                    if j >= 1:
                        attv(psyA, V_aug[:, j - 1, 2 * p, :], ptA_tiles[j - 1], j - 1)
                    if j == 3 and prevB is None and rbA is not None:
                        evac_mult(p - 1, 0, tmpA_prev[0], rbA)
                        tmpA_prev = rbA = None
                    # previous pair's head-B att@V: 2 chunks per j on js 0-3
                    if prevB is not None and j < 4:
                        for jj in (2 * j, 2 * j + 1):
                            attv(prevB, V_aug[:, jj, 2 * (p - 1) + 1, :],
                                 ptB_prev[jj], jj)
                        if j == 3:
                            if rbA is not None:
                                evac_mult(p - 1, 0, tmpA_prev[0], rbA)
                                tmpA_prev = rbA = None
                            tmpB_prev = evac_copy(p - 1, 1, prevB)
                            rbB = evac_recip(p - 1, 1, tmpB_prev[1])
                    if j == 6 and prevB is not None:
                        evac_mult(p - 1, 1, tmpB_prev[0], rbB)
                    # next pair's projections on js 4-7 (PSUM slot freed above)
                    if p + 1 < NP:
                        if j == 4:
                            qps = psX.tile([P, 1024], f32, tag="aux", name="psQ")
                            emit_qk_mms(p + 1, "Q", qps, range(0, 3))
                        elif j == 5:
                            emit_qk_mms(p + 1, "Q", qps, range(3, KS))
                            qt_nxt = emit_qk_evac(p + 1, "Q", qps)
                        elif j == 6:
                            kps = psX.tile([P, 1024], f32, tag="aux", name="psK")
                            emit_qk_mms(p + 1, "K", kps, range(0, 3))
                        elif j == 7:
                            emit_qk_mms(p + 1, "K", kps, range(3, KS))
                            kt_nxt = emit_qk_evac(p + 1, "K", kps)
                    elif last:
                        # pair 5: head-B att@V inline on js 4-7 once the aux
                        # slot is free (accumulation order over j is free)
                        if j == 4:
                            psyB = psX.tile([P, 1024], f32, tag="aux", name="psyB5")
                        if j >= 4:
                            for jj in (2 * (j - 4), 2 * (j - 4) + 1):
                                attv(psyB, V_aug[:, jj, 2 * p + 1, :],
                                     ptB_tiles[jj], jj)

                attv(psyA, V_aug[:, TS - 1, 2 * p, :], ptA_tiles[TS - 1], TS - 1)
                psyA_prev = psyA
                ptB_prev = list(ptB_tiles)
                qt_cur, kt_cur = qt_nxt, kt_nxt

            # tail: last pair's evacuations
            tmpA_prev = evac_copy(npairs - 1, 0, psyA_prev)
            rbA = evac_recip(npairs - 1, 0, tmpA_prev[1])
            if psyB is not None:
                tmpB_prev = evac_copy(NP - 1, 1, psyB)
                rbB = evac_recip(NP - 1, 1, tmpB_prev[1])
            evac_mult(npairs - 1, 0, tmpA_prev[0], rbA)
            if psyB is not None:
                evac_mult(NP - 1, 1, tmpB_prev[0], rbB)

            if phases < 4:
                return

            # Wp load (deferred; reuses Wq's SBUF)
            Wr["p"] = const.tile([P, KS, C], bf16, tag="Wq", name="Wpr")
            nc.sync.dma_start(Wr["p"][:], W_r["p"])

            # ---- output projection ----
            for ts_ in range(TS):
                po = psS.tile([P, 1024], f32, tag=f"S{ts_ % 2}", name="po")
                for k in range(KS):
                    lhsT = YT[:, k, ts_ * P:(ts_ + 1) * P]
                    nc.tensor.matmul(po[:, 0:512], lhsT, Wr["p"][:, k, 0:512],
                                     start=(k == 0), stop=(k == KS - 1))
                    nc.tensor.matmul(po[:, 512:768], lhsT, Wr["p"][:, k, 512:768],
                                     start=(k == 0), stop=(k == KS - 1))
                ot = opool.tile([P, C], f32, tag=f"ot{ts_ % 2}", name="ot")
                nc.vector.tensor_tensor(ot[:], po[:, 0:768], bpB[:], op=ALU.add)
                nc.scalar.dma_start(y_d[ts_ * P:(ts_ + 1) * P, :], ot[:])

        if reps == 1:
            body()
        else:
            import concourse.mybir as _mb
            with tc.For_i(0, reps, 1, hint_engines=tuple(_mb.ALL_ENGINES)):
                body()

    nc.compile()
    return nc


class _Runner:
    """Compile once, run many times on the 8 axon-tunneled cores via PJRT."""

    def __init__(self, nc, n_cores):
        import jax
        import concourse.mybir as mybir
        from jax.sharding import Mesh, PartitionSpec
        from jax.experimental.shard_map import shard_map
        from concourse.bass2jax import (
            _bass_exec_p, install_neuronx_cc_hook, partition_id_tensor)

        install_neuronx_cc_hook()
        self.jax = jax
        self.n_cores = n_cores
        partition_name = nc.partition_id_tensor.name if nc.partition_id_tensor else None
        in_names, out_names, out_avals, zero_outs = [], [], [], []
        for alloc in nc.m.functions[0].allocations:
            if not isinstance(alloc, mybir.MemoryLocationSet):
                continue
            name = alloc.memorylocations[0].name
            if alloc.kind == "ExternalInput":
                if name != partition_name:
                    in_names.append(name)
            elif alloc.kind == "ExternalOutput":
                shape = tuple(alloc.tensor_shape)
                dtype = mybir.dt.np(alloc.dtype)
                out_names.append(name)
                out_avals.append(jax.core.ShapedArray(shape, dtype))
                zero_outs.append(np.zeros(shape, dtype))
        self.in_names, self.out_names = in_names, out_names
        self.zero_outs = zero_outs
        all_in = list(in_names) + list(out_names)
        if partition_name is not None:
            all_in.append(partition_name)

        def _body(*args):
            operands = list(args)
            if partition_name is not None:
                operands.append(partition_id_tensor())
            return tuple(_bass_exec_p.bind(
                *operands, out_avals=tuple(out_avals), in_names=tuple(all_in),
                out_names=tuple(out_names), lowering_input_output_aliases=(),
                sim_require_finite=True, sim_require_nnan=True, nc=nc))

        devices = jax.devices()[:n_cores]
        self.mesh = Mesh(np.asarray(devices), ("core",))
        spec = PartitionSpec("core")
        self.fn = jax.jit(
            shard_map(_body, mesh=self.mesh,
                      in_specs=(spec,) * (len(in_names) + len(out_names)),
                      out_specs=(spec,) * len(out_names), check_rep=False),
            keep_unused=True)

    def stage(self, in_maps):
        import jax
        from jax.sharding import PartitionSpec
        concat = [
            np.concatenate([np.asarray(in_maps[c][n]) for c in range(self.n_cores)], axis=0)
            for n in self.in_names
        ] + [np.concatenate([z] * self.n_cores, axis=0) for z in self.zero_outs]
        sharding = jax.sharding.NamedSharding(self.mesh, PartitionSpec("core"))
        return [jax.device_put(a, sharding) for a in concat]

    def run(self, staged):
        outs = self.fn(*staged)
        self.jax.block_until_ready(outs)
        return outs

    def run_to_maps(self, staged):
        outs = self.run(staged)
        res = []
        for c in range(self.n_cores):
            m = {}
            for i, n in enumerate(self.out_names):
                g = np.asarray(outs[i])
                per = g.shape[0] // self.n_cores
                m[n] = g[c * per:(c + 1) * per]
            res.append(m)
        return res


def get_runner(reps: int = 1, phases: int = 4, npairs: int = NP):
    key = (reps, phases, npairs)
    if key not in _RUNNER_CACHE:
        nc = build_nc(reps, phases, npairs)
        _RUNNER_CACHE[key] = _Runner(nc, N_CORES)
    return _RUNNER_CACHE[key]


def make_in_maps(x, Wq, bq, Wk, bk, Wv, bv, Wp, bp):
    import ml_dtypes
    bf = ml_dtypes.bfloat16
    x = np.asarray(x, dtype=np.float32)
    weights = {
        "Wq": np.asarray(Wq, bf), "Wk": np.asarray(Wk, bf),
        "Wv": np.asarray(Wv, bf), "Wp": np.asarray(Wp, bf),
    }
    bqT = np.ascontiguousarray(np.asarray(bq, np.float32).reshape(KS, P).T)
    bkT = np.ascontiguousarray(np.asarray(bk, np.float32).reshape(KS, P).T)
    bvB = np.ascontiguousarray(np.broadcast_to(np.asarray(bv, np.float32), (P, C)))
    bpB = np.ascontiguousarray(np.broadcast_to(np.asarray(bp, np.float32), (P, C)))
    in_maps = []
    for b in range(B):
        in_maps.append({
            "xT": np.ascontiguousarray(x[b].T).astype(bf),
            "Wq": weights["Wq"], "Wk": weights["Wk"],
            "Wv": weights["Wv"], "Wp": weights["Wp"],
            "bqT": bqT, "bkT": bkT, "bvB": bvB, "bpB": bpB,
        })
    return in_maps


def kernel(x, Wq, bq, Wk, bk, Wv, bv, Wp, bp):
    runner = get_runner(reps=1)
    in_maps = make_in_maps(x, Wq, bq, Wk, bk, Wv, bv, Wp, bp)
    staged = runner.stage(in_maps)
    res = runner.run_to_maps(staged)
    return np.stack([res[b]["y"] for b in range(B)], axis=0)


# revision 3
# speedup vs baseline: 1.0775x; 1.0775x over previous
"""Multi-head attention forward on 8 Trainium2 NeuronCores (Bass/Tile), v2.

Problem: nn_MultiHeadAttention — B=8, T=1024, C=768, H=12, D=64, fp32 in/out.

Sharding: data-parallel over batch — one batch element per core; weights
broadcast (each core loads its own copy). No collectives. Host pre-transposes
x[b] to x^T [C, T] and converts x/W to bf16; fp32 PSUM accumulation keeps the
final rel err ~7e-3 (tolerance 2e-2).

Per-core kernel, all matmul operands bf16 (PE streams 1 col/cycle, same as
f32r, but: FWL weight loads, 2x ScalarE exp via bf16 output, half the DMA/SBUF
footprint):
  1. V = x @ Wv -> V_aug [128, T/128, H, 65] bf16 with a ones column per head
     (the ones row of att@V yields the softmax denominator for free).
  2. Per head pair p: Q^T/K^T chunks [128, T] (pair-major: head A on
     partitions 0-63, head B on 64-127) via matmul(lhsT=W[:,co], rhs=xT).
  3. S^T per pair via ROW-TILED matmuls: tile (0,0) contracts partitions 0-63
     (head A, K=64), tile (64,0) contracts 64-127 (head B) — both run
     CONCURRENTLY on the PE (measured 246ns per 512-col pair vs 430ns for the
     zero-padded K=128 scheme). pt = exp(S^T/8) on ScalarE with bf16 output
     (measured 560ns vs 1129ns for fp32 output on [128,1024]).
  4. att@V: Ytil[65, i] += V_aug[:, j, h, :]^T @ pt chunks. Head A consumes
     pt_A immediately; head B's pt is kept in SBUF and its att@V runs during
     the NEXT pair's (ScalarE-bound) S/exp phase, time-sharing its PSUM slot
     with the next pair's Q/K projection (PSUM: 4 banks S^T + 2 psyA + 2
     shared = exactly 8).
  5. Normalization at Ytil evacuation: denominator row -> ScalarE copy to a
     partition-0 staging row, DVE reciprocal_approx_fast, GpSimd
     partition_broadcast across the 64 d-rows, one DVE multiply into YT
     (bf16). The output projection then only needs a plain bias add.
"""
import numpy as np

B, T, C = 8, 1024, 768
H, D = 12, 64
P = 128
KS = C // P          # 6 contraction subtiles
TS = T // P          # 8 t subtiles
NP = H // 2          # 6 head pairs
N_CORES = 8

_RUNNER_CACHE = {}


def build_nc(reps: int = 1, phases: int = 4, npairs: int = NP):
    import concourse.bacc as bacc
    import concourse.mybir as mybir
    import concourse.tile as tile
    from contextlib import ExitStack

    f32 = mybir.dt.float32
    bf16 = mybir.dt.bfloat16
    AF = mybir.ActivationFunctionType
    ALU = mybir.AluOpType

    nc = bacc.Bacc(num_devices=N_CORES)

    xT_d = nc.dram_tensor("xT", [C, T], bf16, kind="ExternalInput")
    W_d = {w: nc.dram_tensor(f"W{w}", [C, C], bf16, kind="ExternalInput")
           for w in ("q", "k", "v", "p")}
    bqT_d = nc.dram_tensor("bqT", [P, KS], f32, kind="ExternalInput")
    bkT_d = nc.dram_tensor("bkT", [P, KS], f32, kind="ExternalInput")
    bvB_d = nc.dram_tensor("bvB", [P, C], f32, kind="ExternalInput")
    bpB_d = nc.dram_tensor("bpB", [P, C], f32, kind="ExternalInput")
    y_d = nc.dram_tensor("y", [T, C], f32, kind="ExternalOutput")

    with tile.TileContext(nc) as tc, ExitStack() as ctx:
        const = ctx.enter_context(tc.tile_pool(name="const", bufs=1))
        qkp = ctx.enter_context(tc.tile_pool(name="qk", bufs=1))
        ptAp = ctx.enter_context(tc.tile_pool(name="ptA", bufs=1))
        ptBp = ctx.enter_context(tc.tile_pool(name="ptB", bufs=1))
        opool = ctx.enter_context(tc.tile_pool(name="out", bufs=1))
        psS = ctx.enter_context(tc.tile_pool(name="psS", bufs=1, space="PSUM"))
        psY = ctx.enter_context(tc.tile_pool(name="psY", bufs=1, space="PSUM"))
        psX = ctx.enter_context(tc.tile_pool(name="psX", bufs=1, space="PSUM"))

        def body(_iv=None):
            # ---- loads ----
            xTr = const.tile([P, KS, T], bf16, tag="xT", name="xTr")
            Wr = {}
            for w in ("q", "k", "v"):
                Wr[w] = const.tile([P, KS, C], bf16, tag=f"W{w}", name=f"W{w}r")
            xT_r = xT_d.rearrange("(ks p) t -> p ks t", p=P)
            W_r = {w: W_d[w].rearrange("(ks p) c -> p ks c", p=P)
                   for w in ("q", "k", "v", "p")}
            # one DMA per tensor: each dma_start has ~1-2us fixed completion
            # latency on the HWDGE ring, so few big transfers beat many slices.
            # Big tensors first (they gate V-proj/QK-proj), biases after.
            nc.sync.dma_start(Wr["v"][:], W_r["v"])
            nc.sync.dma_start(xTr[:], xT_r)
            bvB = const.tile([P, C], f32, tag="bvB", name="bvB")
            nc.sync.dma_start(bvB[:], bvB_d[:, :])
            nc.sync.dma_start(Wr["q"][:], W_r["q"])
            nc.sync.dma_start(Wr["k"][:], W_r["k"])
            bqT = const.tile([P, KS], f32, tag="bqT", name="bqT")
            nc.sync.dma_start(bqT[:], bqT_d[:, :])
            bkT = const.tile([P, KS], f32, tag="bkT", name="bkT")
            nc.sync.dma_start(bkT[:], bkT_d[:, :])
            bpB = const.tile([P, C], f32, tag="bpB", name="bpB")
            nc.sync.dma_start(bpB[:], bpB_d[:, :])

            ones1 = const.tile([P, 1], f32, tag="ones", name="ones1")
            nc.vector.memset(ones1[:], 1.0)

            # ---- V projection into V_aug (bf16) with ones column ----
            V_aug = const.tile([P, TS, H, D + 1], bf16, tag="Vaug", name="Vaug")
            nc.vector.tensor_copy(V_aug[:, :, :, D:D + 1],
                                  ones1[:].to_broadcast([P, TS, H, 1]))
            for ts_ in range(TS):
                psv = psS.tile([P, 1024], f32, tag=f"S{ts_ % 2}", name="psv")
                for k in range(KS):
                    lhsT = xTr[:, k, ts_ * P:(ts_ + 1) * P]
                    nc.tensor.matmul(psv[:, 0:512], lhsT, Wr["v"][:, k, 0:512],
                                     start=(k == 0), stop=(k == KS - 1))
                    nc.tensor.matmul(psv[:, 512:768], lhsT, Wr["v"][:, k, 512:768],
                                     start=(k == 0), stop=(k == KS - 1))
                nc.vector.tensor_tensor(
                    V_aug[:, ts_, :, 0:D],
                    psv[:, 0:768].rearrange("p (h d) -> p h d", h=H),
                    bvB[:].rearrange("p (h d) -> p h d", h=H), op=ALU.add)

            if phases < 3:
                YTdummy = opool.tile([P, C], f32, tag="ot", name="ytd")
                nc.vector.memset(YTdummy[:], 0.0)
                nc.sync.dma_start(y_d[0:P, :], YTdummy[:])
                return

            YT = const.tile([P, KS, T], bf16, tag="YTs", name="YT")

            def emit_qk_proj(p, which):
                """Q or K projection for pair p into qkp tile (bf16 [128,T])."""
                w = "q" if which == "Q" else "k"
                bias = bqT if which == "Q" else bkT
                ps = psX.tile([P, 1024], f32, tag="aux", name=f"ps{which}")
                for ih in range(2):
                    for k in range(KS):
                        nc.tensor.matmul(
                            ps[:, ih * 512:(ih + 1) * 512],
                            Wr[w][:, k, p * P:(p + 1) * P],
                            xTr[:, k, ih * 512:(ih + 1) * 512],
                            start=(k == 0), stop=(k == KS - 1))
                out = qkp.tile([P, T], bf16, tag=f"{which}{p % 2}", name=f"{which}T2")
                nc.vector.tensor_tensor(
                    out[:], ps[:], bias[:, p:p + 1].to_broadcast([P, T]),
                    op=ALU.add)
                return out

            def evac_copy(p, hh, psy):
                """Fast PSUM->SBUF evacuation: Ytil rows via DVE, denominator
                row via ScalarE to a partition-0 staging row (custom-DVE recip
                needs base partition 0). These are the only psy readers, so the
                accumulator banks free after ~1.2us instead of the whole
                normalize chain."""
                h = 2 * p + hh
                tmp = qkp.tile([P, T], f32, tag=f"tmp{hh}{p % 2}", name="tmp")
                nc.vector.tensor_copy(tmp[0:D, :], psy[0:D, :])
                dst = qkp.tile([1, T], f32, tag=f"dst{h % 2}", name="dstage")
                nc.scalar.copy(dst[:], psy[D:D + 1, :])
                return tmp, dst

            def evac_recip(p, hh, dst):
                """Reciprocal + partition-broadcast of the denominator row; the
                GpSimd broadcast (~1.5us) runs while the PE continues, and the
                dependent multiply is emitted much later so the DVE FIFO never
                blocks on it."""
                h = 2 * p + hh
                rcp = qkp.tile([1, T], f32, tag=f"rcp{h % 2}", name="rcp")
                nc.vector.reciprocal_approx_fast(rcp[:], dst[:])
                rb = qkp.tile([D, T], f32, tag=f"rb{h % 2}", name="rb")
                nc.gpsimd.partition_broadcast(rb[:], rcp[:])
                return rb

            def evac_mult(p, hh, tmp, rb):
                b0 = 64 * hh
                nc.vector.tensor_tensor(YT[b0:b0 + 64, p, :], tmp[0:D, :], rb[:],
                                        op=ALU.mult)

            # ---- attention pair loop (software-pipelined) ----
            # Per j step the PE emits: S^T(j) [4 row-tiled MMs], attV_A(j-1)
            # [2 MMs, one j behind so exp(j-1) is long done], 2 chunks of the
            # PREVIOUS pair's head-B att@V on js 0-3 (its PSUM slot then hands
            # over to the next pair's Q/K projection MMs on js 4-7). The PE
            # never FIFO-blocks on a fresh exp result.
            def emit_qk_mms(p, which, ps, kslice):
                w = "q" if which == "Q" else "k"
                for ih in range(2):
                    for k in kslice:
                        nc.tensor.matmul(
                            ps[:, ih * 512:(ih + 1) * 512],
                            Wr[w][:, k, p * P:(p + 1) * P],
                            xTr[:, k, ih * 512:(ih + 1) * 512],
                            start=(k == 0), stop=(k == KS - 1))

            def emit_qk_evac(p, which, ps):
                bias = bqT if which == "Q" else bkT
                out = qkp.tile([P, T], bf16, tag=f"{which}{p % 2}", name=f"{which}T2")
                nc.vector.tensor_tensor(
                    out[:], ps[:], bias[:, p:p + 1].to_broadcast([P, T]),
                    op=ALU.add)
                return out

            def emit_qk_proj(p, which):
                ps = psX.tile([P, 1024], f32, tag="aux", name=f"ps{which}")
                emit_qk_mms(p, which, ps, range(KS))
                return emit_qk_evac(p, which, ps)

            def attv(psy, v_slice, pt, j):
                for ih in range(2):
                    sl = slice(ih * 512, (ih + 1) * 512)
                    nc.tensor.matmul(psy[0:D + 1, sl], v_slice, pt[:, sl],
                                     start=(j == 0), stop=(j == TS - 1))

            qt_cur = emit_qk_proj(0, "Q")
            kt_cur = emit_qk_proj(0, "K")
            tmpA_prev = tmpB_prev = None
            psyA_prev = None
            qt_nxt = kt_nxt = None
            ptB_tiles = [None] * TS
            ptB_prev = [None] * TS

            for p in range(npairs):
                last = (p == NP - 1)
                # free the previous pair's accumulator with one fast copy,
                # then normalize off the critical path
                if psyA_prev is not None:
                    tmpA_prev = evac_copy(p - 1, 0, psyA_prev)
                psyA = psY.tile([P, 1024], f32, tag="psyA", name="psyA")
                rbA = None
                if tmpA_prev is not None:
                    rbA = evac_recip(p - 1, 0, tmpA_prev[1])

                prevB = None
                if p > 0:
                    prevB = psX.tile([P, 1024], f32, tag="aux", name="psyBprev")
                psyB = None
                ptA_tiles = [None] * TS

                qps = kps = None
                for j in range(TS):
                    psA = psS.tile([P, 1024], f32, tag="S0", name="psA")
                    psB = psS.tile([P, 1024], f32, tag="S1", name="psB")
                    for ih in range(2):
                        sl = slice(ih * 512, (ih + 1) * 512)
                        nc.tensor.matmul(psA[:, sl],
                                         kt_cur[0:64, j * P:(j + 1) * P],
                                         qt_cur[0:64, sl],
                                         start=True, stop=True,
                                         tile_position=(0, 0))
                        nc.tensor.matmul(psB[:, sl],
                                         kt_cur[64:128, j * P:(j + 1) * P],
                                         qt_cur[64:128, sl],
                                         start=True, stop=True,
                                         tile_position=(64, 0))
                    ptA = ptAp.tile([P, 1024], bf16, tag=f"ptA{j % 4}", name="ptA")
                    nc.scalar.activation(ptA[:], psA[:], AF.Exp, scale=0.125)
                    ptA_tiles[j] = ptA
                    ptB = ptBp.tile([P, 1024], bf16, tag=f"ptB{j}", name="ptB")
                    nc.scalar.activation(ptB[:], psB[:], AF.Exp, scale=0.125)
                    ptB_tiles[j] = ptB

                    # attV_A one j behind: exp(j-1) finished during S(j)
                    if j >= 1:
                        attv(psyA, V_aug[:, j - 1, 2 * p, :], ptA_tiles[j - 1], j - 1)
                    # previous pair's head-B att@V: 3/3/2 chunks on js 0-2,
                    # so its PSUM slot frees early for the next projections
                    if prevB is not None and j < 3:
                        for jj in range(3 * j, min(3 * j + 3, TS)):
                            attv(prevB, V_aug[:, jj, 2 * (p - 1) + 1, :],
                                 ptB_prev[jj], jj)
                        if j == 2:
                            tmpB_prev = evac_copy(p - 1, 1, prevB)
                            rbB = evac_recip(p - 1, 1, tmpB_prev[1])
                    if j == 3 and rbA is not None:
                        evac_mult(p - 1, 0, tmpA_prev[0], rbA)
                        tmpA_prev = rbA = None
                    if j == 6 and prevB is not None:
                        evac_mult(p - 1, 1, tmpB_prev[0], rbB)
                    # next pair's projections on js 3-6: the KT evacuation
                    # lands before the pair boundary, off the S^T(p+1, 0) path
                    if p + 1 < NP:
                        if j == 3:
                            qps = psX.tile([P, 1024], f32, tag="aux", name="psQ")
                            emit_qk_mms(p + 1, "Q", qps, range(0, 3))
                        elif j == 4:
                            emit_qk_mms(p + 1, "Q", qps, range(3, KS))
                            qt_nxt = emit_qk_evac(p + 1, "Q", qps)
                        elif j == 5:
                            kps = psX.tile([P, 1024], f32, tag="aux", name="psK")
                            emit_qk_mms(p + 1, "K", kps, range(0, 3))
                        elif j == 6:
                            emit_qk_mms(p + 1, "K", kps, range(3, KS))
                            kt_nxt = emit_qk_evac(p + 1, "K", kps)
                    elif last:
                        # pair 5: head-B att@V inline on js 4-7 once the aux
                        # slot is free (accumulation order over j is free)
                        if j == 4:
                            psyB = psX.tile([P, 1024], f32, tag="aux", name="psyB5")
                        if j >= 4:
                            for jj in (2 * (j - 4), 2 * (j - 4) + 1):
                                attv(psyB, V_aug[:, jj, 2 * p + 1, :],
                                     ptB_tiles[jj], jj)

                attv(psyA, V_aug[:, TS - 1, 2 * p, :], ptA_tiles[TS - 1], TS - 1)
                psyA_prev = psyA
                ptB_prev = list(ptB_tiles)
                qt_cur, kt_cur = qt_nxt, kt_nxt

            # tail: last pair's evacuations
            tmpA_prev = evac_copy(npairs - 1, 0, psyA_prev)
            rbA = evac_recip(npairs - 1, 0, tmpA_prev[1])
            if psyB is not None:
                tmpB_prev = evac_copy(NP - 1, 1, psyB)
                rbB = evac_recip(NP - 1, 1, tmpB_prev[1])
            evac_mult(npairs - 1, 0, tmpA_prev[0], rbA)
            if psyB is not None:
                evac_mult(NP - 1, 1, tmpB_prev[0], rbB)

            if phases < 4:
                return

            # Wp load (deferred; reuses Wq's SBUF)
            Wr["p"] = const.tile([P, KS, C], bf16, tag="Wq", name="Wpr")
            nc.sync.dma_start(Wr["p"][:], W_r["p"])

            # ---- output projection ----
            for ts_ in range(TS):
                po = psS.tile([P, 1024], f32, tag=f"S{ts_ % 2}", name="po")
                for k in range(KS):
                    lhsT = YT[:, k, ts_ * P:(ts_ + 1) * P]
                    nc.tensor.matmul(po[:, 0:512], lhsT, Wr["p"][:, k, 0:512],
                                     start=(k == 0), stop=(k == KS - 1))
                    nc.tensor.matmul(po[:, 512:768], lhsT, Wr["p"][:, k, 512:768],
                                     start=(k == 0), stop=(k == KS - 1))
                ot = opool.tile([P, C], f32, tag=f"ot{ts_ % 2}", name="ot")
                nc.vector.tensor_tensor(ot[:], po[:, 0:768], bpB[:], op=ALU.add)
                nc.scalar.dma_start(y_d[ts_ * P:(ts_ + 1) * P, :], ot[:])

        if reps == 1:
            body()
        else:
            import concourse.mybir as _mb
            with tc.For_i(0, reps, 1, hint_engines=tuple(_mb.ALL_ENGINES)):
                body()

    nc.compile()
    return nc


class _Runner:
    """Compile once, run many times on the 8 axon-tunneled cores via PJRT."""

    def __init__(self, nc, n_cores):
        import jax
        import concourse.mybir as mybir
        from jax.sharding import Mesh, PartitionSpec
        from jax.experimental.shard_map import shard_map
        from concourse.bass2jax import (
            _bass_exec_p, install_neuronx_cc_hook, partition_id_tensor)

        install_neuronx_cc_hook()
        self.jax = jax
        self.n_cores = n_cores
        partition_name = nc.partition_id_tensor.name if nc.partition_id_tensor else None
        in_names, out_names, out_avals, zero_outs = [], [], [], []
        for alloc in nc.m.functions[0].allocations:
            if not isinstance(alloc, mybir.MemoryLocationSet):
                continue
            name = alloc.memorylocations[0].name
            if alloc.kind == "ExternalInput":
                if name != partition_name:
                    in_names.append(name)
            elif alloc.kind == "ExternalOutput":
                shape = tuple(alloc.tensor_shape)
                dtype = mybir.dt.np(alloc.dtype)
                out_names.append(name)
                out_avals.append(jax.core.ShapedArray(shape, dtype))
                zero_outs.append(np.zeros(shape, dtype))
        self.in_names, self.out_names = in_names, out_names
        self.zero_outs = zero_outs
        all_in = list(in_names) + list(out_names)
        if partition_name is not None:
            all_in.append(partition_name)

        def _body(*args):
            operands = list(args)
            if partition_name is not None:
                operands.append(partition_id_tensor())
            return tuple(_bass_exec_p.bind(
                *operands, out_avals=tuple(out_avals), in_names=tuple(all_in),
                out_names=tuple(out_names), lowering_input_output_aliases=(),
                sim_require_finite=True, sim_require_nnan=True, nc=nc))

        devices = jax.devices()[:n_cores]
        self.mesh = Mesh(np.asarray(devices), ("core",))
        spec = PartitionSpec("core")
        self.fn = jax.jit(
            shard_map(_body, mesh=self.mesh,
                      in_specs=(spec,) * (len(in_names) + len(out_names)),
                      out_specs=(spec,) * len(out_names), check_rep=False),
            keep_unused=True)

    def stage(self, in_maps):
        import jax
        from jax.sharding import PartitionSpec
        concat = [
            np.concatenate([np.asarray(in_maps[c][n]) for c in range(self.n_cores)], axis=0)
            for n in self.in_names
        ] + [np.concatenate([z] * self.n_cores, axis=0) for z in self.zero_outs]
        sharding = jax.sharding.NamedSharding(self.mesh, PartitionSpec("core"))
        return [jax.device_put(a, sharding) for a in concat]

    def run(self, staged):
        outs = self.fn(*staged)
        self.jax.block_until_ready(outs)
        return outs

    def run_to_maps(self, staged):
        outs = self.run(staged)
        res = []
        for c in range(self.n_cores):
            m = {}
            for i, n in enumerate(self.out_names):
                g = np.asarray(outs[i])
                per = g.shape[0] // self.n_cores
                m[n] = g[c * per:(c + 1) * per]
            res.append(m)
        return res


def get_runner(reps: int = 1, phases: int = 4, npairs: int = NP):
    key = (reps, phases, npairs)
    if key not in _RUNNER_CACHE:
        nc = build_nc(reps, phases, npairs)
        _RUNNER_CACHE[key] = _Runner(nc, N_CORES)
    return _RUNNER_CACHE[key]


def make_in_maps(x, Wq, bq, Wk, bk, Wv, bv, Wp, bp):
    import ml_dtypes
    bf = ml_dtypes.bfloat16
    x = np.asarray(x, dtype=np.float32)
    weights = {
        "Wq": np.asarray(Wq, bf), "Wk": np.asarray(Wk, bf),
        "Wv": np.asarray(Wv, bf), "Wp": np.asarray(Wp, bf),
    }
    bqT = np.ascontiguousarray(np.asarray(bq, np.float32).reshape(KS, P).T)
    bkT = np.ascontiguousarray(np.asarray(bk, np.float32).reshape(KS, P).T)
    bvB = np.ascontiguousarray(np.broadcast_to(np.asarray(bv, np.float32), (P, C)))
    bpB = np.ascontiguousarray(np.broadcast_to(np.asarray(bp, np.float32), (P, C)))
    in_maps = []
    for b in range(B):
        in_maps.append({
            "xT": np.ascontiguousarray(x[b].T).astype(bf),
            "Wq": weights["Wq"], "Wk": weights["Wk"],
            "Wv": weights["Wv"], "Wp": weights["Wp"],
            "bqT": bqT, "bkT": bkT, "bvB": bvB, "bpB": bpB,
        })
    return in_maps


def kernel(x, Wq, bq, Wk, bk, Wv, bv, Wp, bp):
    runner = get_runner(reps=1)
    in_maps = make_in_maps(x, Wq, bq, Wk, bk, Wv, bv, Wp, bp)
    staged = runner.stage(in_maps)
    res = runner.run_to_maps(staged)
    return np.stack([res[b]["y"] for b in range(B)], axis=0)


# revision 5
# speedup vs baseline: 1.3978x; 1.2973x over previous
"""Multi-head attention forward on 8 Trainium2 NeuronCores (Bass/Tile), v2.

Problem: nn_MultiHeadAttention — B=8, T=1024, C=768, H=12, D=64, fp32 in/out.

Sharding: data-parallel over batch — one batch element per core; weights
broadcast (each core loads its own copy). No collectives. Host pre-transposes
x[b] to x^T [C, T] and converts x/W to bf16; fp32 PSUM accumulation keeps the
final rel err ~7e-3 (tolerance 2e-2).

Per-core kernel, all matmul operands bf16 (PE streams 1 col/cycle, same as
f32r, but: FWL weight loads, 2x ScalarE exp via bf16 output, half the DMA/SBUF
footprint):
  1. V = x @ Wv -> V_aug [128, T/128, H, 65] bf16 with a ones column per head
     (the ones row of att@V yields the softmax denominator for free).
  2. Per head pair p: Q^T/K^T chunks [128, T] (pair-major: head A on
     partitions 0-63, head B on 64-127) via matmul(lhsT=W[:,co], rhs=xT).
  3. S^T per pair via ROW-TILED matmuls: tile (0,0) contracts partitions 0-63
     (head A, K=64), tile (64,0) contracts 64-127 (head B) — both run
     CONCURRENTLY on the PE (measured 246ns per 512-col pair vs 430ns for the
     zero-padded K=128 scheme). pt = exp(S^T/8) on ScalarE with bf16 output
     (measured 560ns vs 1129ns for fp32 output on [128,1024]).
  4. att@V: Ytil[65, i] += V_aug[:, j, h, :]^T @ pt chunks, software-
     pipelined one j-step behind the exp so the PE FIFO never blocks on a
     fresh ScalarE result. Head A consumes pt_A immediately; head B's pt is
     kept in SBUF and its att@V runs early in the NEXT pair's j-loop (js 0-2),
     after which its PSUM slot hands over to the next pair's Q/K projections
     (js 3-6). PSUM: 4 banks S^T + 2 psyA + 2 time-shared = exactly 8.
     Output DMAs go on the scalar-engine HWDGE ring so the next iteration's
     input loads (sync ring) are not queued behind them.
  5. Normalization at Ytil evacuation: denominator row -> ScalarE copy to a
     partition-0 staging row, DVE reciprocal_approx_fast, GpSimd
     partition_broadcast across the 64 d-rows, one DVE multiply into YT
     (bf16). The output projection then only needs a plain bias add.
"""
import numpy as np

B, T, C = 8, 1024, 768
H, D = 12, 64
P = 128
KS = C // P          # 6 contraction subtiles
TS = T // P          # 8 t subtiles
NP = H // 2          # 6 head pairs
N_CORES = 8

_RUNNER_CACHE = {}


def build_nc(reps: int = 1, phases: int = 4, npairs: int = NP):
    import concourse.bacc as bacc
    import concourse.mybir as mybir
    import concourse.tile as tile
    from contextlib import ExitStack

    f32 = mybir.dt.float32
    bf16 = mybir.dt.bfloat16
    AF = mybir.ActivationFunctionType
    ALU = mybir.AluOpType

    nc = bacc.Bacc(num_devices=N_CORES)

    xT_d = nc.dram_tensor("xT", [C, T], bf16, kind="ExternalInput")
    W_d = {w: nc.dram_tensor(f"W{w}", [C, C], bf16, kind="ExternalInput")
           for w in ("q", "k", "v", "p")}
    bqT_d = nc.dram_tensor("bqT", [P, KS], f32, kind="ExternalInput")
    bkT_d = nc.dram_tensor("bkT", [P, KS], f32, kind="ExternalInput")
    bvB_d = nc.dram_tensor("bvB", [P, C], f32, kind="ExternalInput")
    bpB_d = nc.dram_tensor("bpB", [P, C], f32, kind="ExternalInput")
    y_d = nc.dram_tensor("y", [T, C], f32, kind="ExternalOutput")

    with tile.TileContext(nc) as tc, ExitStack() as ctx:
        const = ctx.enter_context(tc.tile_pool(name="const", bufs=1))
        qkp = ctx.enter_context(tc.tile_pool(name="qk", bufs=1))
        ptAp = ctx.enter_context(tc.tile_pool(name="ptA", bufs=1))
        ptBp = ctx.enter_context(tc.tile_pool(name="ptB", bufs=1))
        opool = ctx.enter_context(tc.tile_pool(name="out", bufs=1))
        psS = ctx.enter_context(tc.tile_pool(name="psS", bufs=1, space="PSUM"))
        psY = ctx.enter_context(tc.tile_pool(name="psY", bufs=1, space="PSUM"))
        psX = ctx.enter_context(tc.tile_pool(name="psX", bufs=1, space="PSUM"))

        def body(_iv=None):
            # ---- loads ----
            xTr = const.tile([P, KS, T], bf16, tag="xT", name="xTr")
            Wr = {}
            for w in ("q", "k", "v"):
                Wr[w] = const.tile([P, KS, C], bf16, tag=f"W{w}", name=f"W{w}r")
            xT_r = xT_d.rearrange("(ks p) t -> p ks t", p=P)
            W_r = {w: W_d[w].rearrange("(ks p) c -> p ks c", p=P)
                   for w in ("q", "k", "v", "p")}
            # one DMA per tensor: each dma_start has ~1-2us fixed completion
            # latency on the HWDGE ring, so few big transfers beat many slices.
            # Big tensors first (they gate V-proj/QK-proj), biases after.
            nc.sync.dma_start(Wr["v"][:], W_r["v"])
            nc.sync.dma_start(xTr[:], xT_r)
            bvB = const.tile([P, C], f32, tag="bvB", name="bvB")
            nc.sync.dma_start(bvB[:], bvB_d[:, :])
            nc.sync.dma_start(Wr["q"][:], W_r["q"])
            nc.sync.dma_start(Wr["k"][:], W_r["k"])
            bqT = const.tile([P, KS], f32, tag="bqT", name="bqT")
            nc.sync.dma_start(bqT[:], bqT_d[:, :])
            bkT = const.tile([P, KS], f32, tag="bkT", name="bkT")
            nc.sync.dma_start(bkT[:], bkT_d[:, :])
            bpB = const.tile([P, C], f32, tag="bpB", name="bpB")
            nc.sync.dma_start(bpB[:], bpB_d[:, :])

            ones1 = const.tile([P, 1], f32, tag="ones", name="ones1")
            nc.vector.memset(ones1[:], 1.0)

            # ---- V projection into V_aug (bf16) with ones column ----
            V_aug = const.tile([P, TS, H, D + 1], bf16, tag="Vaug", name="Vaug")
            nc.vector.tensor_copy(V_aug[:, :, :, D:D + 1],
                                  ones1[:].to_broadcast([P, TS, H, 1]))
            for ts_ in range(TS):
                psv = (psY.tile([P, 1024], f32, tag="psyA", name="psv")
                       if ts_ % 2 == 0 else
                       psX.tile([P, 1024], f32, tag="aux", name="psv"))
                for k in range(KS):
                    lhsT = xTr[:, k, ts_ * P:(ts_ + 1) * P]
                    nc.tensor.matmul(psv[:, 0:512], lhsT, Wr["v"][:, k, 0:512],
                                     start=(k == 0), stop=(k == KS - 1))
                    nc.tensor.matmul(psv[:, 512:768], lhsT, Wr["v"][:, k, 512:768],
                                     start=(k == 0), stop=(k == KS - 1))
                nc.vector.tensor_tensor(
                    V_aug[:, ts_, :, 0:D],
                    psv[:, 0:768].rearrange("p (h d) -> p h d", h=H),
                    bvB[:].rearrange("p (h d) -> p h d", h=H), op=ALU.add)

            if phases < 3:
                YTdummy = opool.tile([P, C], f32, tag="ot", name="ytd")
                nc.vector.memset(YTdummy[:], 0.0)
                nc.sync.dma_start(y_d[0:P, :], YTdummy[:])
                return

            YT = const.tile([P, KS, T], bf16, tag="YTs", name="YT")

            def emit_qk_proj(p, which):
                """Q or K projection for pair p into qkp tile (bf16 [128,T])."""
                w = "q" if which == "Q" else "k"
                bias = bqT if which == "Q" else bkT
                ps = psX.tile([P, 1024], f32, tag="aux", name=f"ps{which}")
                for ih in range(2):
                    for k in range(KS):
                        nc.tensor.matmul(
                            ps[:, ih * 512:(ih + 1) * 512],
                            Wr[w][:, k, p * P:(p + 1) * P],
                            xTr[:, k, ih * 512:(ih + 1) * 512],
                            start=(k == 0), stop=(k == KS - 1))
                out = qkp.tile([P, T], bf16, tag=f"{which}{p % 2}", name=f"{which}T2")
                nc.vector.tensor_tensor(
                    out[:], ps[:], bias[:, p:p + 1].to_broadcast([P, T]),
                    op=ALU.add)
                return out

            def evac_copy(p, hh, psy):
                """Fast PSUM->SBUF evacuation: Ytil rows via DVE, denominator
                row via ScalarE to a partition-0 staging row (custom-DVE recip
                needs base partition 0). These are the only psy readers, so the
                accumulator banks free after ~1.2us instead of the whole
                normalize chain."""
                h = 2 * p + hh
                tmp = qkp.tile([P, T], f32, tag=f"tmp{hh}{p % 2}", name="tmp")
                nc.vector.tensor_copy(tmp[0:D, :], psy[0:D, :])
                dst = qkp.tile([1, T], f32, tag=f"dst{h % 2}", name="dstage")
                nc.scalar.copy(dst[:], psy[D:D + 1, :])
                return tmp, dst

            def evac_recip(p, hh, dst):
                """Reciprocal + partition-broadcast of the denominator row; the
                GpSimd broadcast (~1.5us) runs while the PE continues, and the
                dependent multiply is emitted much later so the DVE FIFO never
                blocks on it."""
                h = 2 * p + hh
                rcp = qkp.tile([1, T], f32, tag=f"rcp{h % 2}", name="rcp")
                nc.vector.reciprocal_approx_fast(rcp[:], dst[:])
                rb = qkp.tile([D, T], f32, tag=f"rb{h % 2}", name="rb")
                nc.gpsimd.partition_broadcast(rb[:], rcp[:])
                return rb

            def evac_mult(p, hh, tmp, rb):
                b0 = 64 * hh
                nc.vector.tensor_tensor(YT[b0:b0 + 64, p, :], tmp[0:D, :], rb[:],
                                        op=ALU.mult)

            # ---- attention pair loop (software-pipelined) ----
            # Per j step the PE emits: S^T(j) [4 row-tiled MMs], attV_A(j-1)
            # [2 MMs, one j behind so exp(j-1) is long done], 2 chunks of the
            # PREVIOUS pair's head-B att@V on js 0-3 (its PSUM slot then hands
            # over to the next pair's Q/K projection MMs on js 4-7). The PE
            # never FIFO-blocks on a fresh exp result.
            def emit_qk_mms(p, which, ps, kslice):
                w = "q" if which == "Q" else "k"
                for ih in range(2):
                    for k in kslice:
                        nc.tensor.matmul(
                            ps[:, ih * 512:(ih + 1) * 512],
                            Wr[w][:, k, p * P:(p + 1) * P],
                            xTr[:, k, ih * 512:(ih + 1) * 512],
                            start=(k == 0), stop=(k == KS - 1))

            def emit_qk_evac(p, which, ps):
                bias = bqT if which == "Q" else bkT
                out = qkp.tile([P, T], bf16, tag=f"{which}{p % 2}", name=f"{which}T2")
                nc.vector.tensor_tensor(
                    out[:], ps[:], bias[:, p:p + 1].to_broadcast([P, T]),
                    op=ALU.add)
                return out

            def emit_qk_proj(p, which):
                ps = psX.tile([P, 1024], f32, tag="aux", name=f"ps{which}")
                emit_qk_mms(p, which, ps, range(KS))
                return emit_qk_evac(p, which, ps)

            def attv(psy, v_slice, pt, j, half=0):
                for ih in range(2):
                    sl = slice(ih * 512, (ih + 1) * 512)
                    src = slice(half * 1024 + ih * 512, half * 1024 + (ih + 1) * 512)
                    nc.tensor.matmul(psy[0:D + 1, sl], v_slice, pt[:, src],
                                     start=(j == 0), stop=(j == TS - 1))

            qt_cur = emit_qk_proj(0, "Q")
            kt_cur = emit_qk_proj(0, "K")
            tmpA_prev = tmpB_prev = None
            psyA_prev = None
            qt_nxt = kt_nxt = None
            ptB_tiles = [None] * TS
            ptB_prev = [None] * TS

            for p in range(npairs):
                last = (p == NP - 1)
                # free the previous pair's accumulator with one fast copy,
                # then normalize off the critical path
                if psyA_prev is not None:
                    tmpA_prev = evac_copy(p - 1, 0, psyA_prev)
                psyA = psY.tile([P, 1024], f32, tag="psyA", name="psyA")
                rbA = None
                if tmpA_prev is not None:
                    rbA = evac_recip(p - 1, 0, tmpA_prev[1])

                prevB = None
                if p > 0:
                    prevB = psX.tile([P, 1024], f32, tag="aux", name="psyBprev")
                psyB = None
                ptA_tiles = [None] * TS

                qps = kps = None
                for j in range(TS):
                    psAB = psS.tile([P, 2048], f32, tag="S", name="psAB")
                    for ih in range(2):
                        sl = slice(ih * 512, (ih + 1) * 512)
                        slB = slice(1024 + ih * 512, 1024 + (ih + 1) * 512)
                        nc.tensor.matmul(psAB[:, sl],
                                         kt_cur[0:64, j * P:(j + 1) * P],
                                         qt_cur[0:64, sl],
                                         start=True, stop=True,
                                         tile_position=(0, 0))
                        nc.tensor.matmul(psAB[:, slB],
                                         kt_cur[64:128, j * P:(j + 1) * P],
                                         qt_cur[64:128, sl],
                                         start=True, stop=True,
                                         tile_position=(64, 0))
                    pt2 = ptBp.tile([P, 2048], bf16, tag=f"pt{j}", name="pt2")
                    nc.scalar.activation(pt2[:], psAB[:], AF.Exp, scale=0.125)
                    ptA_tiles[j] = pt2
                    ptB_tiles[j] = pt2

                    # attV_A one j behind: exp(j-1) finished during S(j)
                    if j >= 1:
                        attv(psyA, V_aug[:, j - 1, 2 * p, :], ptA_tiles[j - 1], j - 1)
                    # previous pair's head-B att@V: 3/3/2 chunks on js 0-2,
                    # so its PSUM slot frees early for the next projections
                    if prevB is not None and j < 3:
                        for jj in range(3 * j, min(3 * j + 3, TS)):
                            attv(prevB, V_aug[:, jj, 2 * (p - 1) + 1, :],
                                 ptB_prev[jj], jj, half=1)
                        if j == 2:
                            tmpB_prev = evac_copy(p - 1, 1, prevB)
                            rbB = evac_recip(p - 1, 1, tmpB_prev[1])
                    if j == 3 and rbA is not None:
                        evac_mult(p - 1, 0, tmpA_prev[0], rbA)
                        tmpA_prev = rbA = None
                    if j == 6 and prevB is not None:
                        evac_mult(p - 1, 1, tmpB_prev[0], rbB)
                    # next pair's projections on js 3-6: the KT evacuation
                    # lands before the pair boundary, off the S^T(p+1, 0) path
                    if p + 1 < NP:
                        if j == 3:
                            qps = psX.tile([P, 1024], f32, tag="aux", name="psQ")
                            emit_qk_mms(p + 1, "Q", qps, range(0, 3))
                        elif j == 4:
                            emit_qk_mms(p + 1, "Q", qps, range(3, KS))
                            qt_nxt = emit_qk_evac(p + 1, "Q", qps)
                        elif j == 5:
                            kps = psX.tile([P, 1024], f32, tag="aux", name="psK")
                            emit_qk_mms(p + 1, "K", kps, range(0, 3))
                        elif j == 6:
                            emit_qk_mms(p + 1, "K", kps, range(3, KS))
                            kt_nxt = emit_qk_evac(p + 1, "K", kps)
                    elif last:
                        # pair 5: head-B att@V inline on js 4-7 once the aux
                        # slot is free (accumulation order over j is free)
                        if j == 4:
                            psyB = psX.tile([P, 1024], f32, tag="aux", name="psyB5")
                        if j >= 4:
                            for jj in (2 * (j - 4), 2 * (j - 4) + 1):
                                attv(psyB, V_aug[:, jj, 2 * p + 1, :],
                                     ptB_tiles[jj], jj, half=1)

                attv(psyA, V_aug[:, TS - 1, 2 * p, :], ptA_tiles[TS - 1], TS - 1)
                psyA_prev = psyA
                ptB_prev = list(ptB_tiles)
                qt_cur, kt_cur = qt_nxt, kt_nxt

            # tail: last pair's evacuations
            tmpA_prev = evac_copy(npairs - 1, 0, psyA_prev)
            rbA = evac_recip(npairs - 1, 0, tmpA_prev[1])
            if psyB is not None:
                tmpB_prev = evac_copy(NP - 1, 1, psyB)
                rbB = evac_recip(NP - 1, 1, tmpB_prev[1])
            evac_mult(npairs - 1, 0, tmpA_prev[0], rbA)
            if psyB is not None:
                evac_mult(NP - 1, 1, tmpB_prev[0], rbB)

            if phases < 4:
                return

            # Wp load (deferred; reuses Wq's SBUF)
            Wr["p"] = const.tile([P, KS, C], bf16, tag="Wq", name="Wpr")
            nc.sync.dma_start(Wr["p"][:], W_r["p"])

            # ---- output projection ----
            for ts_ in range(TS):
                po2 = psS.tile([P, 2048], f32, tag="S", name="po2") if ts_ % 2 == 0 else po2
                po = po2[:, 0:1024] if ts_ % 2 == 0 else po2[:, 1024:2048]
                for k in range(KS):
                    lhsT = YT[:, k, ts_ * P:(ts_ + 1) * P]
                    nc.tensor.matmul(po[:, 0:512], lhsT, Wr["p"][:, k, 0:512],
                                     start=(k == 0), stop=(k == KS - 1))
                    nc.tensor.matmul(po[:, 512:768], lhsT, Wr["p"][:, k, 512:768],
                                     start=(k == 0), stop=(k == KS - 1))
                ot = opool.tile([P, C], f32, tag=f"ot{ts_ % 2}", name="ot")
                nc.vector.tensor_tensor(ot[:], po[:, 0:768], bpB[:], op=ALU.add)
                nc.scalar.dma_start(y_d[ts_ * P:(ts_ + 1) * P, :], ot[:])

        if reps == 1:
            body()
        else:
            import concourse.mybir as _mb
            with tc.For_i(0, reps, 1, hint_engines=tuple(_mb.ALL_ENGINES)):
                body()

    nc.compile()
    return nc


class _Runner:
    """Compile once, run many times on the 8 axon-tunneled cores via PJRT."""

    def __init__(self, nc, n_cores):
        import jax
        import concourse.mybir as mybir
        from jax.sharding import Mesh, PartitionSpec
        from jax.experimental.shard_map import shard_map
        from concourse.bass2jax import (
            _bass_exec_p, install_neuronx_cc_hook, partition_id_tensor)

        install_neuronx_cc_hook()
        self.jax = jax
        self.n_cores = n_cores
        partition_name = nc.partition_id_tensor.name if nc.partition_id_tensor else None
        in_names, out_names, out_avals, zero_outs = [], [], [], []
        for alloc in nc.m.functions[0].allocations:
            if not isinstance(alloc, mybir.MemoryLocationSet):
                continue
            name = alloc.memorylocations[0].name
            if alloc.kind == "ExternalInput":
                if name != partition_name:
                    in_names.append(name)
            elif alloc.kind == "ExternalOutput":
                shape = tuple(alloc.tensor_shape)
                dtype = mybir.dt.np(alloc.dtype)
                out_names.append(name)
                out_avals.append(jax.core.ShapedArray(shape, dtype))
                zero_outs.append(np.zeros(shape, dtype))
        self.in_names, self.out_names = in_names, out_names
        self.zero_outs = zero_outs
        all_in = list(in_names) + list(out_names)
        if partition_name is not None:
            all_in.append(partition_name)

        def _body(*args):
            operands = list(args)
            if partition_name is not None:
                operands.append(partition_id_tensor())
            return tuple(_bass_exec_p.bind(
                *operands, out_avals=tuple(out_avals), in_names=tuple(all_in),
                out_names=tuple(out_names), lowering_input_output_aliases=(),
                sim_require_finite=True, sim_require_nnan=True, nc=nc))

        devices = jax.devices()[:n_cores]
        self.mesh = Mesh(np.asarray(devices), ("core",))
        spec = PartitionSpec("core")
        self.fn = jax.jit(
            shard_map(_body, mesh=self.mesh,
                      in_specs=(spec,) * (len(in_names) + len(out_names)),
                      out_specs=(spec,) * len(out_names), check_rep=False),
            keep_unused=True)

    def stage(self, in_maps):
        import jax
        from jax.sharding import PartitionSpec
        concat = [
            np.concatenate([np.asarray(in_maps[c][n]) for c in range(self.n_cores)], axis=0)
            for n in self.in_names
        ] + [np.concatenate([z] * self.n_cores, axis=0) for z in self.zero_outs]
        sharding = jax.sharding.NamedSharding(self.mesh, PartitionSpec("core"))
        return [jax.device_put(a, sharding) for a in concat]

    def run(self, staged):
        outs = self.fn(*staged)
        self.jax.block_until_ready(outs)
        return outs

    def run_to_maps(self, staged):
        outs = self.run(staged)
        res = []
        for c in range(self.n_cores):
            m = {}
            for i, n in enumerate(self.out_names):
                g = np.asarray(outs[i])
                per = g.shape[0] // self.n_cores
                m[n] = g[c * per:(c + 1) * per]
            res.append(m)
        return res


def get_runner(reps: int = 1, phases: int = 4, npairs: int = NP):
    key = (reps, phases, npairs)
    if key not in _RUNNER_CACHE:
        nc = build_nc(reps, phases, npairs)
        _RUNNER_CACHE[key] = _Runner(nc, N_CORES)
    return _RUNNER_CACHE[key]


def make_in_maps(x, Wq, bq, Wk, bk, Wv, bv, Wp, bp):
    import ml_dtypes
    bf = ml_dtypes.bfloat16
    x = np.asarray(x, dtype=np.float32)
    weights = {
        "Wq": np.asarray(Wq, bf), "Wk": np.asarray(Wk, bf),
        "Wv": np.asarray(Wv, bf), "Wp": np.asarray(Wp, bf),
    }
    bqT = np.ascontiguousarray(np.asarray(bq, np.float32).reshape(KS, P).T)
    bkT = np.ascontiguousarray(np.asarray(bk, np.float32).reshape(KS, P).T)
    bvB = np.ascontiguousarray(np.broadcast_to(np.asarray(bv, np.float32), (P, C)))
    bpB = np.ascontiguousarray(np.broadcast_to(np.asarray(bp, np.float32), (P, C)))
    in_maps = []
    for b in range(B):
        in_maps.append({
            "xT": np.ascontiguousarray(x[b].T).astype(bf),
            "Wq": weights["Wq"], "Wk": weights["Wk"],
            "Wv": weights["Wv"], "Wp": weights["Wp"],
            "bqT": bqT, "bkT": bkT, "bvB": bvB, "bpB": bpB,
        })
    return in_maps


def kernel(x, Wq, bq, Wk, bk, Wv, bv, Wp, bp):
    runner = get_runner(reps=1)
    in_maps = make_in_maps(x, Wq, bq, Wk, bk, Wv, bv, Wp, bp)
    staged = runner.stage(in_maps)
    res = runner.run_to_maps(staged)
    return np.stack([res[b]["y"] for b in range(B)], axis=0)
